# revision 48
# baseline (speedup 1.0000x reference)
"""Trainium2 Bass kernel for DeformConvTranspose1d.

Problem (hardcoded): B=8, Cin=256, Win=4096, Cout=256, K=4, stride=2, pad=1,
out_pad=0, dil=1, groups=1, offset_groups=1 -> Wout=8192.

Math:
  cols[b,co,k,i] = sum_ci x[b,ci,i] * weight[ci,co,k]
  pos = i*2 - 1 + k + offset[b,k,i]
  out[b,co,j] = bias[co] + sum_{k,i} cols[b,co,k,i] * mask[b,k,i] * hat(j - pos)
  where hat(u) = max(0, 1 - |u|)   (linear-interp scatter == hat kernel)

Strategy: data-parallel over batch, 1 sample per NeuronCore (8 cores).
Per core, a software-pipelined loop over 32 chunks of 128 input positions
(matmul operands bf16, PSUM accumulation fp32). Iteration c emits chunk c's
GEMM1 + val copies and chunk c-1's scatter + accumulate, so the val copy is
off the GEMM->scatter critical path and PE stays dense:
  - GEMM1 (TensorE): cols_T[i, (k,co)] = x_chunk^T @ W, two 1-bank PSUM
    tiles of 512 cols so two chunks pipeline in PSUM
  - val = cols PSUM->SBUF bf16 cast (VectorE; GPSIMD cannot touch PSUM)
  - hat build over a WLOC=268-column local output window:
      ScalarE:  u' = |m*jl - m*pos|      (Abs activation, 2 ptr scalars)
      GpSimd:   s  = min(u'-m, 0) = -m*hat   (negated hat weights)
  - scatter matmul (TensorE): po[co, jl] -= sum_k val_k^T @ s_k, one
    1-bank PSUM tile per ch half
  - accumulate window into persistent fp16 out_sb, subtracting po (fixes
    the negation) with bias folded into the first write of each column:
      fresh ch0 on ScalarE (Identity, scale=-1, bias ptr), ch1 on VectorE,
      overlap (16 cols shared with previous window) on VectorE
    stream finished blocks to DRAM as fp16 (host upcasts to f32; halves
    output bytes over the link).
All index arithmetic (transposes, -m*pos bias terms) is host-side numpy, so
every DMA is contiguous per partition (no gather descriptors).
Assumes |offset| < R=5 (offsets are N(0,1); max over this input ~4.9).
"""

import numpy as np

P = 128
B = 8
CIN = 256
WIN = 4096
CO = 256
K = 4
R = 5
WLOC = 268
OVL = WLOC - 256  # 16
NAUX = 10  # 4 negpos + 4 mask + 2 bias
N_CORES = 8

_nc_cache = {}


def build_nc(win=WIN, n_cores=N_CORES):
    import concourse.tile as tile
    from concourse import bacc, mybir

    f32 = mybir.dt.float32
    f16 = mybir.dt.float16
    bf16 = mybir.dt.bfloat16
    Alu = mybir.AluOpType
    Act = mybir.ActivationFunctionType

    nch = win // P
    wout = (win - 1) * 2 - 2 + 3 + 1

    nc = bacc.Bacc("TRN2", target_bir_lowering=False, debug=False,
                   num_devices=n_cores)
    x_d = nc.dram_tensor("x", [P, 2, win], bf16, kind="ExternalInput")
    w_d = nc.dram_tensor("wr", [P, 2, K * CO], bf16, kind="ExternalInput")
    aux_d = nc.dram_tensor("aux", [P, nch, NAUX], f32, kind="ExternalInput")
    out_d = nc.dram_tensor("out", [CO, wout], f16, kind="ExternalOutput")

    with tile.TileContext(nc) as tc:
        with (
            tc.tile_pool(name="const", bufs=1) as constp,
            tc.tile_pool(name="outp", bufs=1) as outp,
            tc.tile_pool(name="val", bufs=4) as valp,
            tc.tile_pool(name="ubuf", bufs=4) as ubp,
            tc.tile_pool(name="sbuf_s", bufs=4) as sp,
            tc.tile_pool(name="pcols", bufs=2, space="PSUM") as pcols,
            tc.tile_pool(name="pout", bufs=2, space="PSUM") as poutp,
        ):
            aux_sb = constp.tile([P, nch, NAUX], f32)
            nc.sync.dma_start(out=aux_sb[:], in_=aux_d.ap())
            x_sb = constp.tile([P, 2, win], bf16)
            xcuts = [0, 128, 1472, 2816, win]
            nc.sync.dma_start(out=x_sb[:, :, 0:128], in_=x_d.ap()[:, :, 0:128])
            w_sb = constp.tile([P, 2, K * CO], bf16)
            nc.sync.dma_start(out=w_sb[:], in_=w_d.ap())
            for q in range(1, 4):
                nc.sync.dma_start(out=x_sb[:, :, xcuts[q]:xcuts[q + 1]],
                                  in_=x_d.ap()[:, :, xcuts[q]:xcuts[q + 1]])
            iota_f = constp.tile([P, WLOC], f32)
            nc.gpsimd.iota(iota_f[:], pattern=[[1, WLOC]], base=0,
                           channel_multiplier=0,
                           allow_small_or_imprecise_dtypes=True)
            zeros_b = constp.tile([P, WLOC], bf16)
            nc.vector.memset(zeros_b[:], 0)
            out_sb = outp.tile([P, 2, wout], f16)

            # Software-pipelined: iteration c emits chunk c's GEMM1 + val
            # copies and chunk c-1's scatter + accumulate, so val copies get
            # a full GEMM-phase to land and PE stays dense.
            dma_done = 0
            prev = None
            for c in range(nch + 1):
                if c < nch:
                    # u' = |m*jl - m*pos| on Act (the only abs engine)
                    u_all = ubp.tile([P, K, WLOC], bf16)
                    for k in range(K):
                        nc.scalar.activation(out=u_all[:, k, :],
                                             in_=iota_f[:], func=Act.Abs,
                                             bias=aux_sb[:, c, k:k + 1],
                                             scale=aux_sb[:, c, 4 + k:5 + k])
                    # s = min(u'-m, 0) = -m*hat on GpSimd (SBUF-only)
                    s_all = sp.tile([P, K, WLOC], bf16)
                    for k in range(K):
                        nc.gpsimd.tensor_scalar(
                            out=s_all[:, k, :], in0=u_all[:, k, :],
                            scalar1=aux_sb[:, c, 4 + k:5 + k],
                            scalar2=0.0, op0=Alu.subtract, op1=Alu.min)
                    # val copy PSUM->SBUF bf16 (VectorE, per 512-block)
                    val_sb = valp.tile([P, K * CO], bf16)
                    for n in range(2):
                        cols_ps = pcols.tile([P, 512], f32, name=f"cols{n}")
                        for h in range(2):
                            nc.tensor.matmul(
                                out=cols_ps[:],
                                lhsT=x_sb[:, h, c * P:(c + 1) * P],
                                rhs=w_sb[:, h, n * 512:(n + 1) * 512],
                                start=(h == 0), stop=(h == 1))
                        nc.vector.tensor_copy(
                            val_sb[:, n * 512:(n + 1) * 512], cols_ps[:])
                    cur = (val_sb, s_all)
                else:
                    cur = None
                if prev is not None:
                    pc = c - 1
                    val_p, s_p = prev
                    po = [poutp.tile([P, WLOC], f32, name=f"po{ch}")
                          for ch in range(2)]
                    for k in range(K):
                        for ch in range(2):
                            lo = k * CO + ch * P
                            nc.tensor.matmul(out=po[ch][:],
                                             lhsT=val_p[:, lo:lo + P],
                                             rhs=s_p[:, k, :],
                                             start=(k == 0),
                                             stop=(k == K - 1))
                    jbase = 256 * pc - 1 - R
                    if pc == 0:
                        fsl = (slice(0, WLOC - 1 - R), slice(1 + R, WLOC))
                    else:
                        # overlap: out -= po (po is negated contribution)
                        for ch in range(2):
                            nc.vector.tensor_tensor(
                                out=out_sb[:, ch, jbase:jbase + OVL],
                                in0=out_sb[:, ch, jbase:jbase + OVL],
                                in1=po[ch][:, 0:OVL], op=Alu.subtract)
                        fe = min(jbase + WLOC, wout)
                        fsl = (slice(jbase + OVL, fe),
                               slice(OVL, OVL + (fe - (jbase + OVL))))
                    # fresh: out = bias - po (ch0 on Act, ch1 on DVE)
                    wfr = fsl[0].stop - fsl[0].start
                    nc.scalar.activation(
                        out=out_sb[:, 0, fsl[0]], in_=po[0][:, fsl[1]],
                        func=Act.Identity, scale=-1.0,
                        bias=aux_sb[:, 0, 8:9])
                    nc.vector.tensor_tensor(
                        out=out_sb[:, 1, fsl[0]],
                        in0=aux_sb[:, 0, 9:10].to_broadcast([P, wfr]),
                        in1=po[1][:, fsl[1]], op=Alu.subtract)
                    if pc in (7, 15, 23, 29, 30, 31):
                        end = wout if pc == nch - 1 else 256 * (pc + 1) - 1 - R
                        if pc == nch - 1:
                            # final store: one DMA for both ch halves
                            nc.sync.dma_start(
                                out=out_d.ap()
                                    .rearrange("(h p) w -> p h w", p=P)
                                    [:, :, dma_done:end],
                                in_=out_sb[:, :, dma_done:end])
                        else:
                            for ch in range(2):
                                nc.sync.dma_start(
                                    out=out_d.ap()[ch * P:(ch + 1) * P,
                                                   dma_done:end],
                                    in_=out_sb[:, ch, dma_done:end])
                        dma_done = end
                prev = cur
    nc.compile()
    return nc


def _get_nc():
    key = (WIN, N_CORES)
    if key not in _nc_cache:
        _nc_cache[key] = build_nc(WIN, N_CORES)
    return _nc_cache[key]


def make_in_maps(x, weight, offset, mask, bias, win=WIN):
    import ml_dtypes
    bf = ml_dtypes.bfloat16
    nB = x.shape[0]
    nch = win // P
    # weight [Cin, Cout, K] -> wr[p, h, k*CO+co] with ci = h*128 + p
    wr = np.ascontiguousarray(
        np.transpose(weight, (0, 2, 1)).reshape(2, P, K * CO)
        .transpose(1, 0, 2)).astype(bf)
    iota_p = np.arange(P, dtype=np.float32)
    iota_k = np.arange(K, dtype=np.float32)
    # negpos[p, c, k] = -(2p + k + R) - offset[k, c*128+p]
    base = -(2.0 * iota_p[None, :, None, None] +
             iota_k[None, None, None, :] + R)
    # [B, K, win] -> [B, p, c, k]
    offT = offset.transpose(0, 2, 1).reshape(nB, nch, P, K).transpose(
        0, 2, 1, 3)
    mT = mask.transpose(0, 2, 1).reshape(nB, nch, P, K).transpose(0, 2, 1, 3)
    aux_all = np.zeros((nB, P, nch, NAUX), dtype=np.float32)
    aux_all[:, :, :, 0:4] = mT * (base - offT)
    aux_all[:, :, :, 4:8] = mT
    aux_all[:, :, 0, 8] = bias[None, :P]
    aux_all[:, :, 0, 9] = bias[None, P:]
    xp_all = np.ascontiguousarray(
        x.reshape(nB, 2, P, win).transpose(0, 2, 1, 3)).astype(bf)
    return [{"x": xp_all[b], "wr": wr, "aux": aux_all[b]}
            for b in range(nB)]


TRACE = False
last_results = None


def kernel(x, weight, offset, mask, bias):
    global last_results
    from concourse.bass_utils import run_bass_kernel_spmd

    x = np.asarray(x, dtype=np.float32)
    weight = np.asarray(weight, dtype=np.float32)
    offset = np.asarray(offset, dtype=np.float32)
    mask = np.asarray(mask, dtype=np.float32)
    bias = np.asarray(bias, dtype=np.float32)

    nc = _get_nc()
    in_maps = make_in_maps(x, weight, offset, mask, bias)
    res = run_bass_kernel_spmd(nc, in_maps, core_ids=list(range(N_CORES)),
                               trace=TRACE)
    last_results = res
    return np.stack([res.results[b]["out"] for b in range(B)]).astype(
        np.float32)


# revision 49
# speedup vs baseline: 1.0011x; 1.0011x over previous
"""Trainium2 Bass kernel for DeformConvTranspose1d.

Problem (hardcoded): B=8, Cin=256, Win=4096, Cout=256, K=4, stride=2, pad=1,
out_pad=0, dil=1, groups=1, offset_groups=1 -> Wout=8192.

Math:
  cols[b,co,k,i] = sum_ci x[b,ci,i] * weight[ci,co,k]
  pos = i*2 - 1 + k + offset[b,k,i]
  out[b,co,j] = bias[co] + sum_{k,i} cols[b,co,k,i] * mask[b,k,i] * hat(j - pos)
  where hat(u) = max(0, 1 - |u|)   (linear-interp scatter == hat kernel)

Strategy: data-parallel over batch, 1 sample per NeuronCore (8 cores).
Per core, a software-pipelined loop over 32 chunks of 128 input positions
(matmul operands bf16, PSUM accumulation fp32). Iteration c emits chunk c's
GEMM1 + val copies and chunk c-1's scatter + accumulate, so the val copy is
off the GEMM->scatter critical path and PE stays dense:
  - GEMM1 (TensorE): cols_T[i, (k,co)] = x_chunk^T @ W, two 1-bank PSUM
    tiles of 512 cols so two chunks pipeline in PSUM
  - val = cols PSUM->SBUF bf16 cast (VectorE; GPSIMD cannot touch PSUM)
  - hat build over a WLOC=268-column local output window:
      ScalarE:  u' = |m*jl - m*pos|      (Abs activation, 2 ptr scalars)
      GpSimd:   s  = min(u'-m, 0) = -m*hat   (negated hat weights)
  - scatter matmul (TensorE): po[co, jl] -= sum_k val_k^T @ s_k, one
    1-bank PSUM tile per ch half
  - accumulate window into persistent fp16 out_sb, subtracting po (fixes
    the negation) with bias folded into the first write of each column:
      fresh ch0 on ScalarE (Identity, scale=-1, bias ptr), ch1 on VectorE,
      overlap (16 cols shared with previous window) on VectorE
    stream finished blocks to DRAM as fp16 (host upcasts to f32; halves
    output bytes over the link).
All index arithmetic (transposes, -m*pos bias terms) is host-side numpy, so
every DMA is contiguous per partition (no gather descriptors).
Assumes |offset| < R=5 (offsets are N(0,1); max over this input ~4.9).
"""

import numpy as np

P = 128
B = 8
CIN = 256
WIN = 4096
CO = 256
K = 4
R = 5
WLOC = 268
OVL = WLOC - 256  # 16
NAUX = 10  # 4 negpos + 4 mask + 2 bias
N_CORES = 8

_nc_cache = {}


def build_nc(win=WIN, n_cores=N_CORES):
    import concourse.tile as tile
    from concourse import bacc, mybir

    f32 = mybir.dt.float32
    f16 = mybir.dt.float16
    bf16 = mybir.dt.bfloat16
    Alu = mybir.AluOpType
    Act = mybir.ActivationFunctionType

    nch = win // P
    wout = (win - 1) * 2 - 2 + 3 + 1

    nc = bacc.Bacc("TRN2", target_bir_lowering=False, debug=False,
                   num_devices=n_cores)
    x_d = nc.dram_tensor("x", [P, 2, win], bf16, kind="ExternalInput")
    w_d = nc.dram_tensor("wr", [P, 2, K * CO], bf16, kind="ExternalInput")
    aux_d = nc.dram_tensor("aux", [P, nch, NAUX], f32, kind="ExternalInput")
    out_d = nc.dram_tensor("out", [CO, wout], f16, kind="ExternalOutput")

    with tile.TileContext(nc) as tc:
        with (
            tc.tile_pool(name="const", bufs=1) as constp,
            tc.tile_pool(name="outp", bufs=1) as outp,
            tc.tile_pool(name="val", bufs=4) as valp,
            tc.tile_pool(name="ubuf", bufs=4) as ubp,
            tc.tile_pool(name="sbuf_s", bufs=4) as sp,
            tc.tile_pool(name="pcols", bufs=2, space="PSUM") as pcols,
            tc.tile_pool(name="pout", bufs=2, space="PSUM") as poutp,
        ):
            aux_sb = constp.tile([P, nch, NAUX], f32)
            nc.sync.dma_start(out=aux_sb[:], in_=aux_d.ap())
            x_sb = constp.tile([P, 2, win], bf16)
            xcuts = [0, 128, 1472, 2816, win]
            nc.sync.dma_start(out=x_sb[:, :, 0:128], in_=x_d.ap()[:, :, 0:128])
            w_sb = constp.tile([P, 2, K * CO], bf16)
            nc.sync.dma_start(out=w_sb[:], in_=w_d.ap())
            for q in range(1, 4):
                nc.sync.dma_start(out=x_sb[:, :, xcuts[q]:xcuts[q + 1]],
                                  in_=x_d.ap()[:, :, xcuts[q]:xcuts[q + 1]])
            iota_f = constp.tile([P, WLOC], f32)
            nc.gpsimd.iota(iota_f[:], pattern=[[1, WLOC]], base=0,
                           channel_multiplier=0,
                           allow_small_or_imprecise_dtypes=True)
            zeros_b = constp.tile([P, WLOC], bf16)
            nc.vector.memset(zeros_b[:], 0)
            out_sb = outp.tile([P, 2, wout], f16)

            # Software-pipelined: iteration c emits chunk c's GEMM1 + val
            # copies and chunk c-1's scatter + accumulate, so val copies get
            # a full GEMM-phase to land and PE stays dense.
            dma_done = 0
            prev = None
            for c in range(nch + 1):
                if c < nch:
                    # u' = |m*jl - m*pos| on Act (the only abs engine)
                    u_all = ubp.tile([P, K, WLOC], bf16)
                    for k in range(K):
                        nc.scalar.activation(out=u_all[:, k, :],
                                             in_=iota_f[:], func=Act.Abs,
                                             bias=aux_sb[:, c, k:k + 1],
                                             scale=aux_sb[:, c, 4 + k:5 + k])
                    # s = min(u'-m, 0) = -m*hat on GpSimd (SBUF-only)
                    s_all = sp.tile([P, K, WLOC], bf16)
                    for k in range(K):
                        nc.gpsimd.tensor_scalar(
                            out=s_all[:, k, :], in0=u_all[:, k, :],
                            scalar1=aux_sb[:, c, 4 + k:5 + k],
                            scalar2=0.0, op0=Alu.subtract, op1=Alu.min)
                    # val copy PSUM->SBUF bf16 (VectorE, per 512-block)
                    val_sb = valp.tile([P, K * CO], bf16)
                    for n in range(2):
                        cols_ps = pcols.tile([P, 512], f32, name=f"cols{n}")
                        for h in range(2):
                            nc.tensor.matmul(
                                out=cols_ps[:],
                                lhsT=x_sb[:, h, c * P:(c + 1) * P],
                                rhs=w_sb[:, h, n * 512:(n + 1) * 512],
                                start=(h == 0), stop=(h == 1))
                        nc.vector.tensor_copy(
                            val_sb[:, n * 512:(n + 1) * 512], cols_ps[:])
                    cur = (val_sb, s_all)
                else:
                    cur = None
                if prev is not None:
                    pc = c - 1
                    val_p, s_p = prev
                    po = [poutp.tile([P, WLOC], f32, name=f"po{ch}")
                          for ch in range(2)]
                    for k in range(K):
                        for ch in range(2):
                            lo = k * CO + ch * P
                            nc.tensor.matmul(out=po[ch][:],
                                             lhsT=val_p[:, lo:lo + P],
                                             rhs=s_p[:, k, :],
                                             start=(k == 0),
                                             stop=(k == K - 1))
                    jbase = 256 * pc - 1 - R
                    if pc == 0:
                        fsl = (slice(0, WLOC - 1 - R), slice(1 + R, WLOC))
                    else:
                        # overlap: out -= po (po is negated contribution)
                        for ch in range(2):
                            nc.vector.tensor_tensor(
                                out=out_sb[:, ch, jbase:jbase + OVL],
                                in0=out_sb[:, ch, jbase:jbase + OVL],
                                in1=po[ch][:, 0:OVL], op=Alu.subtract)
                        fe = min(jbase + WLOC, wout)
                        fsl = (slice(jbase + OVL, fe),
                               slice(OVL, OVL + (fe - (jbase + OVL))))
                    # fresh: out = bias - po (ch0 on Act, ch1 on DVE)
                    wfr = fsl[0].stop - fsl[0].start
                    nc.scalar.activation(
                        out=out_sb[:, 0, fsl[0]], in_=po[0][:, fsl[1]],
                        func=Act.Identity, scale=-1.0,
                        bias=aux_sb[:, 0, 8:9])
                    nc.vector.tensor_tensor(
                        out=out_sb[:, 1, fsl[0]],
                        in0=aux_sb[:, 0, 9:10].to_broadcast([P, wfr]),
                        in1=po[1][:, fsl[1]], op=Alu.subtract)
                    if pc in (7, 15, 23, 29, 30, 31):
                        end = wout if pc == nch - 1 else 256 * (pc + 1) - 1 - R
                        if pc == nch - 1:
                            # final store: one DMA for both ch halves, from
                            # Act (its stream is idle at the tail)
                            nc.scalar.dma_start(
                                out=out_d.ap()
                                    .rearrange("(h p) w -> p h w", p=P)
                                    [:, :, dma_done:end],
                                in_=out_sb[:, :, dma_done:end])
                        else:
                            for ch in range(2):
                                nc.sync.dma_start(
                                    out=out_d.ap()[ch * P:(ch + 1) * P,
                                                   dma_done:end],
                                    in_=out_sb[:, ch, dma_done:end])
                        dma_done = end
                prev = cur
    nc.compile()
    return nc


def _get_nc():
    key = (WIN, N_CORES)
    if key not in _nc_cache:
        _nc_cache[key] = build_nc(WIN, N_CORES)
    return _nc_cache[key]


def make_in_maps(x, weight, offset, mask, bias, win=WIN):
    import ml_dtypes
    bf = ml_dtypes.bfloat16
    nB = x.shape[0]
    nch = win // P
    # weight [Cin, Cout, K] -> wr[p, h, k*CO+co] with ci = h*128 + p
    wr = np.ascontiguousarray(
        np.transpose(weight, (0, 2, 1)).reshape(2, P, K * CO)
        .transpose(1, 0, 2)).astype(bf)
    iota_p = np.arange(P, dtype=np.float32)
    iota_k = np.arange(K, dtype=np.float32)
    # negpos[p, c, k] = -(2p + k + R) - offset[k, c*128+p]
    base = -(2.0 * iota_p[None, :, None, None] +
             iota_k[None, None, None, :] + R)
    # [B, K, win] -> [B, p, c, k]
    offT = offset.transpose(0, 2, 1).reshape(nB, nch, P, K).transpose(
        0, 2, 1, 3)
    mT = mask.transpose(0, 2, 1).reshape(nB, nch, P, K).transpose(0, 2, 1, 3)
    aux_all = np.zeros((nB, P, nch, NAUX), dtype=np.float32)
    aux_all[:, :, :, 0:4] = mT * (base - offT)
    aux_all[:, :, :, 4:8] = mT
    aux_all[:, :, 0, 8] = bias[None, :P]
    aux_all[:, :, 0, 9] = bias[None, P:]
    xp_all = np.ascontiguousarray(
        x.reshape(nB, 2, P, win).transpose(0, 2, 1, 3)).astype(bf)
    return [{"x": xp_all[b], "wr": wr, "aux": aux_all[b]}
            for b in range(nB)]


TRACE = False
last_results = None


def kernel(x, weight, offset, mask, bias):
    global last_results
    from concourse.bass_utils import run_bass_kernel_spmd

    x = np.asarray(x, dtype=np.float32)
    weight = np.asarray(weight, dtype=np.float32)
    offset = np.asarray(offset, dtype=np.float32)
    mask = np.asarray(mask, dtype=np.float32)
    bias = np.asarray(bias, dtype=np.float32)

    nc = _get_nc()
    in_maps = make_in_maps(x, weight, offset, mask, bias)
    res = run_bass_kernel_spmd(nc, in_maps, core_ids=list(range(N_CORES)),
                               trace=TRACE)
    last_results = res
    return np.stack([res.results[b]["out"] for b in range(B)]).astype(
        np.float32)
